# revision 28
# baseline (speedup 1.0000x reference)
"""Trainium2 Bass kernel for a 4-layer dense-GCN + student-t soft-assignment head.

Reference computation (N=8192, D=512, K=20):
    adj_hat  = adj + I
    deg      = rowsum(adj_hat);  d1 = deg^-1/2
    adj_norm = d1[:,None] * adj_hat * d1[None,:]
    h = x;  h = relu(adj_norm @ (h @ Wl))  x4
    q = student-t soft assignment of h onto 20 cluster centers (v=2)
returns (h, q).

Distribution: row-shard adj/x over the 8 NeuronCores (1024 rows each);
replicate weights + clusters. Per layer each core computes its z = h_loc @ W,
pre-scales rows (the D^-1/2 factors commute into per-owner row scales),
all-gathers the scaled activations (bf16) via ncfw AllGather, and contracts
its rows of adj_hat against the gathered activations on the PE.

Layout: activations are kept feature-major (rT = h^T unscaled) so both
matmuls need no per-layer transposes:
    z   [m,dout]  = rT(lhsT) @ W(rhs)
    yT  [dout,m]  = v(lhsT)  @ adjT(rhs)      (accumulated over all 8192 k)
adj^T is produced once in a prologue (PE transpose-mode, bf16) fused with the
degree computation and stored in a DRAM scratch. The "+I" of adj_hat is
folded into the host-side shard (diagonal block +1).

Comm/compute overlap: each layer's activations are gathered in two
m-half chunks. The output accumulation is split so that the first m-half of
yT finishes one k-chunk early; its relu + z + AllGather for the next layer
then run under the tail of the current layer's matmuls. Layer 0's gather
(of unscaled x@W1, degree-scaled on the receive side) runs entirely under
the prologue's transpose pass.
"""

import numpy as np

N, D, K = 8192, 512, 20
NCORES = 8
CH = N // NCORES          # 1024 rows per core
MT = CH // 128            # 8 m-tiles per core
KT = N // 128             # 64 k-tiles (contraction)
DT = D // 128             # 4 feature tiles
NLAYERS = 4

_CACHE = {}


def _build():
    import concourse.bass as bass  # noqa: F401
    import concourse.bacc as bacc
    import concourse.mybir as mybir
    from concourse import tile
    from concourse import masks
    from contextlib import ExitStack

    f32 = mybir.dt.float32
    bf16 = mybir.dt.bfloat16
    ACT = mybir.ActivationFunctionType

    nc = bacc.Bacc("TRN2", target_bir_lowering=False, debug=False,
                   num_devices=NCORES)

    adj_in = nc.dram_tensor("adj", [CH, N], f32, kind="ExternalInput")
    x_in = nc.dram_tensor("x", [CH, D], f32, kind="ExternalInput")
    w_in = nc.dram_tensor("w", [NLAYERS, D, D], f32, kind="ExternalInput")
    ct_in = nc.dram_tensor("ct", [D, K], f32, kind="ExternalInput")
    csq_in = nc.dram_tensor("csq", [128, K], f32, kind="ExternalInput")
    h_out = nc.dram_tensor("h", [CH, D], f32, kind="ExternalOutput")
    q_out = nc.dram_tensor("q", [CH, K], f32, kind="ExternalOutput")

    adjT = nc.dram_tensor("adjT", [N, CH], bf16)
    agin = [[nc.dram_tensor(f"agin{l}_{h}", [CH // 2, D], bf16)
             for h in range(2)] for l in range(NLAYERS)]
    agout = [[nc.dram_tensor(f"agout{l}_{h}", [N // 2, D], bf16,
                             addr_space="Shared")
              for h in range(2)] for l in range(NLAYERS)]
    dgin = nc.dram_tensor("dgin", [CH, 1], f32)
    dgout = nc.dram_tensor("dgout", [N, 1], f32, addr_space="Shared")

    with tile.TileContext(nc) as tc, ExitStack() as stk:
        const_pool = stk.enter_context(tc.tile_pool(name="const", bufs=1))
        psum = stk.enter_context(tc.tile_pool(name="ps", bufs=8, space="PSUM"))
        work = stk.enter_context(tc.tile_pool(name="work", bufs=3))
        abp = stk.enter_context(tc.tile_pool(name="abp", bufs=5))
        smp = stk.enter_context(tc.tile_pool(name="smp", bufs=5))
        wld = stk.enter_context(tc.tile_pool(name="wld", bufs=2))
        strip = stk.enter_context(tc.tile_pool(name="strip", bufs=2))
        ainp = stk.enter_context(tc.tile_pool(name="ainp", bufs=5))
        rtp = stk.enter_context(tc.tile_pool(name="rtp", bufs=2))
        vpool = stk.enter_context(tc.tile_pool(name="vp", bufs=3))
        vkrd = stk.enter_context(tc.tile_pool(name="vkrd", bufs=3))
        akrd = stk.enter_context(tc.tile_pool(name="akrd", bufs=3))

        adjres = const_pool.tile([128, 3, 4, CH], bf16)
        ident = const_pool.tile([128, 128], f32)
        masks.make_identity(nc, ident[:])
        identb = const_pool.tile([128, 128], bf16)
        masks.make_identity(nc, identb[:])

        # ---- resident weights (bf16) and cluster data ----
        wsb = const_pool.tile([128, NLAYERS, DT, 512], bf16)
        for l in range(NLAYERS):
            for dd in range(DT):
                wt = wld.tile([128, 512], f32, tag="wload")
                nc.sync.dma_start(
                    wt[:], w_in.ap()[l, dd * 128:(dd + 1) * 128, :])
                nc.vector.tensor_copy(wsb[:, l, dd], wt[:])
        ctb = const_pool.tile([128, DT, K], bf16)
        for dd in range(DT):
            ctf = wld.tile([128, K], f32, tag="wload")
            nc.sync.dma_start(ctf[:], ct_in.ap()[dd * 128:(dd + 1) * 128, :])
            nc.vector.tensor_copy(ctb[:, dd], ctf[:])
        csq = const_pool.tile([128, K], f32)
        nc.sync.dma_start(csq[:], csq_in.ap())

        # ---- x -> rT0 (x^T, bf16, feature-major) ----
        rT = rtp.tile([128, DT, CH], bf16, name="rT0")
        for mt in range(MT):
            xt = work.tile([128, D], f32, tag="wk2")
            nc.sync.dma_start(xt[:], x_in.ap()[mt * 128:(mt + 1) * 128, :])
            xb = work.tile([128, D], bf16, tag="wk2")
            nc.scalar.activation(xb[:], xt[:], ACT.Copy)
            ps = psum.tile([128, 512], bf16, tag="ps")
            for dd in range(DT):
                nc.tensor.transpose(ps[:, dd * 128:(dd + 1) * 128],
                                    xb[:, dd * 128:(dd + 1) * 128], identb[:])
            nc.vector.tensor_copy(rT[:, :, mt * 128:(mt + 1) * 128],
                                  ps[:].rearrange("p (j m) -> p j m", j=4))

        # ---- layer 0 z = x @ W1 (unscaled), sent during the prologue ----
        for h in range(2):
            for mt in range(h * MT // 2, (h + 1) * MT // 2):
                zps = psum.tile([128, 512], f32, tag="ps", name=f"z0_{mt}")
                for dd in range(DT):
                    nc.tensor.matmul(zps[:],
                                     rT[:, dd, mt * 128:(mt + 1) * 128],
                                     wsb[:, 0, dd, :],
                                     start=(dd == 0), stop=(dd == DT - 1))
                vt = vpool.tile([128, 512], bf16, tag="v", name=f"v0_{mt}")
                nc.scalar.activation(vt[:], zps[:], ACT.Copy)
                nc.scalar.dma_start(
                    agin[0][h].ap()[(mt - h * MT // 2) * 128:
                                    (mt + 1 - h * MT // 2) * 128, :], vt[:])
            nc.gpsimd.collective_compute(
                "AllGather", mybir.AluOpType.bypass,
                replica_groups=[list(range(NCORES))],
                ins=[agin[0][h].ap()], outs=[agout[0][h].ap()])

        # ---- prologue: degree + adj^T (bf16) into DRAM scratch ----
        dega = const_pool.tile([128, MT], f32)
        nc.vector.memset(dega[:], 0.0)
        RES_SBS = {5: 0, 7: 1, 9: 2}
        for ks in range(16):           # 16 column superblocks (512 wide)
            res = RES_SBS.get(ks)
            if res is None:
                dst = strip.tile([128, 4, CH], bf16)
            else:
                dst = adjres[:, res]
            for tp in range(MT // 2):  # pairs of row strips
                ps = psum.tile([128, 1024], bf16, tag="ps")
                for u in range(2):
                    t = tp * 2 + u
                    ain = ainp.tile([128, 512], f32, tag="ain")
                    nc.sync.dma_start(
                        ain[:],
                        adj_in.ap()[t * 128:(t + 1) * 128,
                                    ks * 512:(ks + 1) * 512])
                    ab = abp.tile([128, 512], bf16, tag="abc")
                    part = smp.tile([128, 1], f32, tag="sm1")
                    nc.scalar.activation(ab[:], ain[:], ACT.Copy)
                    nc.vector.reduce_sum(part[:], ab[:],
                                         axis=mybir.AxisListType.X)
                    nc.vector.tensor_add(dega[:, t:t + 1], dega[:, t:t + 1],
                                         part[:])
                    for j in range(4):
                        nc.tensor.transpose(
                            ps[:, u * 512 + j * 128:u * 512 + (j + 1) * 128],
                            ab[:, j * 128:(j + 1) * 128],
                            identb[:])
                nc.vector.tensor_copy(
                    dst[:, :, tp * 256:(tp + 1) * 256].rearrange(
                        "p j (u m) -> p u j m", u=2),
                    ps[:].rearrange("p (u j m) -> p u j m", u=2, j=4))
            if res is None:
                for j in range(4):
                    nc.scalar.dma_start(
                        adjT.ap()[(ks * 4 + j) * 128:(ks * 4 + j + 1) * 128, :],
                        dst[:, j, :])

        # deg-derived row scales (all per local row, [128, MT] layout)
        dinv = const_pool.tile([128, MT], f32)
        nc.vector.reciprocal(dinv[:], dega[:])
        d1 = const_pool.tile([128, MT], f32)
        nc.scalar.activation(d1[:], dinv[:], ACT.Sqrt)
        m2d1 = const_pool.tile([128, MT], f32)
        nc.vector.tensor_scalar_mul(m2d1[:], d1[:], -2.0)
        # gather degrees; build d1g [128, KT] = rsqrt(deg) for every global node
        nc.scalar.dma_start(dgin.ap().rearrange("(t p) o -> p (t o)", p=128),
                            dega[:])
        nc.gpsimd.collective_compute(
            "AllGather", mybir.AluOpType.bypass,
            replica_groups=[list(range(NCORES))],
            ins=[dgin.ap()], outs=[dgout.ap()])
        degg = const_pool.tile([128, KT], f32)
        nc.sync.dma_start(degg[:],
                          dgout.ap().rearrange("(t p) o -> p (t o)", p=128))
        d1gi = const_pool.tile([128, KT], f32)
        nc.vector.reciprocal(d1gi[:], degg[:])
        d1g = const_pool.tile([128, KT], f32)
        nc.scalar.activation(d1g[:], d1gi[:], ACT.Sqrt)


        def head_mt(mt, rTf):
            rps = psum.tile([128, 512], bf16, tag="ps", name=f"rps{mt}")
            for dd in range(DT):
                nc.tensor.transpose(rps[:, dd * 128:(dd + 1) * 128],
                                    rTf[:, dd, mt * 128:(mt + 1) * 128],
                                    identb[:])
            ht = work.tile([128, D], f32, tag="wk2")
            nc.scalar.activation(ht[:], rps[:], ACT.Copy,
                                 scale=d1[:, mt:mt + 1])
            nc.sync.dma_start(h_out.ap()[mt * 128:(mt + 1) * 128, :], ht[:])
            hsqv = work.tile([128, D], f32, tag="wk2")
            nc.vector.tensor_mul(hsqv[:], ht[:], ht[:])
            hsq = smp.tile([128, 1], f32, tag="sm1")
            nc.vector.reduce_sum(hsq[:], hsqv[:], axis=mybir.AxisListType.X)
            gps = psum.tile([128, 512], f32, tag="ps", name=f"gps{mt}")
            for dd in range(DT):
                nc.tensor.matmul(gps[:, 0:K],
                                 rTf[:, dd, mt * 128:(mt + 1) * 128],
                                 ctb[:, dd, :],
                                 start=(dd == 0), stop=(dd == DT - 1))
            t20 = smp.tile([128, K], f32, tag="sm")
            nc.vector.tensor_scalar(t20[:], gps[:, 0:K], m2d1[:, mt:mt + 1],
                                    None, op0=mybir.AluOpType.mult)
            nc.vector.tensor_add(t20[:], t20[:], csq[:])
            nc.vector.tensor_scalar(t20[:], t20[:], hsq[:], None,
                                    op0=mybir.AluOpType.add)
            nc.vector.tensor_scalar(t20[:], t20[:], 0.5, 1.0,
                                    op0=mybir.AluOpType.mult,
                                    op1=mybir.AluOpType.add)
            rec = smp.tile([128, K], f32, tag="sm")
            nc.vector.reciprocal(rec[:], t20[:])
            sq = smp.tile([128, K], f32, tag="sm")
            nc.scalar.activation(sq[:], rec[:], ACT.Sqrt)
            qun = smp.tile([128, K], f32, tag="sm")
            nc.vector.tensor_mul(qun[:], rec[:], sq[:])
            srow = smp.tile([128, 1], f32, tag="sm1")
            nc.vector.reduce_sum(srow[:], qun[:], axis=mybir.AxisListType.X)
            rinv = smp.tile([128, 1], f32, tag="sm1")
            nc.vector.reciprocal(rinv[:], srow[:])
            qf = smp.tile([128, K], f32, tag="sm")
            nc.vector.tensor_scalar(qf[:], qun[:], rinv[:], None,
                                    op0=mybir.AluOpType.mult)
            nc.sync.dma_start(q_out.ap()[mt * 128:(mt + 1) * 128, :], qf[:])

        # ---- 4 GCN layers ----
        def mm_group(yps, vk, ak, mhs, start, stopg):
            """4 k-tiles of matmuls: yps[dd*2+mh] += vk.T @ ak slices."""
            for t in range(4):
                for dd in range(DT):
                    for mh in mhs:
                        nc.tensor.matmul(
                            yps[dd * 2 + mh][:],
                            vk[:, t, dd * 128:(dd + 1) * 128],
                            ak[:, t, mh * 512:(mh + 1) * 512],
                            start=(start and t == 0),
                            stop=(stopg and t == 3))

        def z_half(l_next, h, rT_cur, scale):
            """z + scaled bf16 send for m-half h feeding layer l_next's gather."""
            for mt in range(h * MT // 2, (h + 1) * MT // 2):
                zps = psum.tile([128, 512], f32, tag="ps",
                                name=f"z{l_next}_{mt}")
                for dd in range(DT):
                    nc.tensor.matmul(zps[:],
                                     rT_cur[:, dd, mt * 128:(mt + 1) * 128],
                                     wsb[:, l_next, dd, :],
                                     start=(dd == 0), stop=(dd == DT - 1))
                vt = vpool.tile([128, 512], bf16, tag="v",
                                name=f"v{l_next}_{mt}")
                nc.scalar.activation(vt[:], zps[:], ACT.Copy,
                                     scale=scale[:, mt:mt + 1])
                nc.scalar.dma_start(
                    agin[l_next][h].ap()[(mt - h * MT // 2) * 128:
                                         (mt + 1 - h * MT // 2) * 128, :],
                    vt[:])
            nc.gpsimd.collective_compute(
                "AllGather", mybir.AluOpType.bypass,
                replica_groups=[list(range(NCORES))],
                ins=[agin[l_next][h].ap()], outs=[agout[l_next][h].ap()])

        for l in range(NLAYERS):
            yps = []
            for _yi in range(8):
                ypt = psum.tile([128, 512], f32, tag="ps", name=f"yps{l}_{_yi}")
                yps.append(ypt)

            def load_vk(g, h, l=l, sfx=""):
                vk = vkrd.tile([128, 4, 512], bf16, tag="vk",
                               name=f"vk{l}_{h}_{g}{sfx}")
                nc.sync.dma_start(
                    vk[:],
                    agout[l][h].ap().rearrange("(r p) d -> p r d", p=128)[
                        :, g * 4:(g + 1) * 4, :])
                if l == 0:
                    for t in range(4):
                        kcol = g * 8 + h * 4 + t
                        nc.vector.tensor_scalar_mul(
                            vk[:, t, :], vk[:, t, :], d1g[:, kcol:kcol + 1])
                return vk

            def load_ak(g, h, l=l, sfx=""):
                ak = akrd.tile([128, 4, CH], bf16, tag="ak",
                               name=f"ak{l}_{h}_{g}{sfx}")
                nc.sync.dma_start(
                    ak[:],
                    adjT.ap().rearrange("(r p) m -> p r m", p=128)[
                        :, g * 8 + h * 4:g * 8 + h * 4 + 4, :])
                return ak

            # k-chunk 0 (first m-half rows of every core): both output halves
            for g in range(8):
                vk = load_vk(g, 0)
                ak = load_ak(g, 0)
                mm_group(yps, vk, ak, (0, 1), start=(g == 0), stopg=False)
            # k-chunk 1: output m-half 0 only
            for g in range(8):
                vk = load_vk(g, 1)
                ak = adjres[:, g - 2] if 2 <= g <= 4 else load_ak(g, 1)
                mm_group(yps, vk, ak, (0,), start=False, stopg=(g == 7))
            # first m-half of yT done: relu + feed next layer's first gather
            rT_next = rtp.tile([128, DT, CH], bf16, name=f"rT{l + 1}")
            for dd in range(DT):
                nc.scalar.activation(rT_next[:, dd, 0:512],
                                     yps[dd * 2 + 0][:], ACT.Relu)
            if l + 1 < NLAYERS:
                z_half(l + 1, 0, rT_next, dinv)
            else:
                for mt in range(4):
                    head_mt(mt, rT_next)
            # k-chunk 1, output m-half 1 (vk re-read; adjT mostly resident)
            for g in range(8):
                vk = load_vk(g, 1, sfx="b")
                ak = adjres[:, g - 2] if 2 <= g <= 4 else load_ak(g, 1, sfx="b")
                mm_group(yps, vk, ak, (1,), start=False, stopg=(g == 7))
            for dd in range(DT):
                nc.scalar.activation(rT_next[:, dd, 512:1024],
                                     yps[dd * 2 + 1][:], ACT.Relu)
            if l + 1 < NLAYERS:
                z_half(l + 1, 1, rT_next, dinv)
            else:
                for mt in range(4, MT):
                    head_mt(mt, rT_next)
            rT = rT_next

    nc.compile()
    return nc


def _get_nc():
    if "nc" not in _CACHE:
        _CACHE["nc"] = _build()
    return _CACHE["nc"]


def make_in_maps(x, adj, w1, w2, w3, w4, cluster):
    w = np.stack([w1, w2, w3, w4]).astype(np.float32)
    ct = np.ascontiguousarray(cluster.T.astype(np.float32))
    csq = np.ascontiguousarray(
        np.broadcast_to((cluster.astype(np.float32) ** 2).sum(axis=1)[None, :],
                        (128, K))).astype(np.float32)
    in_maps = []
    for c in range(NCORES):
        shard = np.array(adj[c * CH:(c + 1) * CH, :], dtype=np.float32)
        idx = np.arange(CH)
        shard[idx, c * CH + idx] += 1.0   # fold adj_hat = adj + I into the shard
        in_maps.append({
            "adj": shard,
            "x": np.ascontiguousarray(x[c * CH:(c + 1) * CH].astype(np.float32)),
            "w": w, "ct": ct, "csq": csq,
        })
    return in_maps


def run(in_maps, trace=False):
    from concourse.bass_utils import run_bass_kernel_spmd
    return run_bass_kernel_spmd(_get_nc(), in_maps, list(range(NCORES)),
                                trace=trace)


def kernel(x, adj, w1, w2, w3, w4, cluster):
    res = run(make_in_maps(x, adj, w1, w2, w3, w4, cluster))
    h = np.concatenate([res.results[c]["h"] for c in range(NCORES)], axis=0)
    q = np.concatenate([res.results[c]["q"] for c in range(NCORES)], axis=0)
    return h, q


# revision 29
# speedup vs baseline: 1.0348x; 1.0348x over previous
"""Trainium2 Bass kernel for a 4-layer dense-GCN + student-t soft-assignment head.

Reference computation (N=8192, D=512, K=20):
    adj_hat  = adj + I
    deg      = rowsum(adj_hat);  d1 = deg^-1/2
    adj_norm = d1[:,None] * adj_hat * d1[None,:]
    h = x;  h = relu(adj_norm @ (h @ Wl))  x4
    q = student-t soft assignment of h onto 20 cluster centers (v=2)
returns (h, q).

Distribution: row-shard adj/x over the 8 NeuronCores (1024 rows each);
replicate weights + clusters. Per layer each core computes its z = h_loc @ W,
pre-scales rows (the D^-1/2 factors commute into per-owner row scales),
all-gathers the scaled activations (bf16) via ncfw AllGather, and contracts
its rows of adj_hat against the gathered activations on the PE.

Layout: activations are kept feature-major (rT = h^T unscaled) so both
matmuls need no per-layer transposes:
    z   [m,dout]  = rT(lhsT) @ W(rhs)
    yT  [dout,m]  = v(lhsT)  @ adjT(rhs)      (accumulated over all 8192 k)
adj^T is produced once in a prologue (PE transpose-mode, bf16) fused with the
degree computation and stored in a DRAM scratch. The "+I" of adj_hat is
folded into the host-side shard (diagonal block +1).

Comm/compute overlap: each layer's activations are gathered in two
m-half chunks. The output accumulation is split so that the first m-half of
yT finishes one k-chunk early; its relu + z + AllGather for the next layer
then run under the tail of the current layer's matmuls. Layer 0's gather
(of unscaled x@W1, degree-scaled on the receive side) runs entirely under
the prologue's transpose pass.
"""

import numpy as np

N, D, K = 8192, 512, 20
NCORES = 8
CH = N // NCORES          # 1024 rows per core
MT = CH // 128            # 8 m-tiles per core
KT = N // 128             # 64 k-tiles (contraction)
DT = D // 128             # 4 feature tiles
NLAYERS = 4

_CACHE = {}


def _build():
    import concourse.bass as bass  # noqa: F401
    import concourse.bacc as bacc
    import concourse.mybir as mybir
    from concourse import tile
    from concourse import masks
    from contextlib import ExitStack

    f32 = mybir.dt.float32
    bf16 = mybir.dt.bfloat16
    ACT = mybir.ActivationFunctionType

    nc = bacc.Bacc("TRN2", target_bir_lowering=False, debug=False,
                   num_devices=NCORES)

    adj_in = nc.dram_tensor("adj", [CH, N], f32, kind="ExternalInput")
    x_in = nc.dram_tensor("x", [CH, D], f32, kind="ExternalInput")
    w_in = nc.dram_tensor("w", [NLAYERS, D, D], f32, kind="ExternalInput")
    ct_in = nc.dram_tensor("ct", [D, K], f32, kind="ExternalInput")
    csq_in = nc.dram_tensor("csq", [128, K], f32, kind="ExternalInput")
    h_out = nc.dram_tensor("h", [CH, D], f32, kind="ExternalOutput")
    q_out = nc.dram_tensor("q", [CH, K], f32, kind="ExternalOutput")

    adjT = nc.dram_tensor("adjT", [N, CH], bf16)
    agin = [[nc.dram_tensor(f"agin{l}_{h}", [CH // 2, D], bf16)
             for h in range(2)] for l in range(NLAYERS)]
    agout = [[nc.dram_tensor(f"agout{l}_{h}", [N // 2, D], bf16,
                             addr_space="Shared")
              for h in range(2)] for l in range(NLAYERS)]
    dgin = nc.dram_tensor("dgin", [CH, 1], f32)
    dgout = nc.dram_tensor("dgout", [N, 1], f32, addr_space="Shared")

    with tile.TileContext(nc) as tc, ExitStack() as stk:
        const_pool = stk.enter_context(tc.tile_pool(name="const", bufs=1))
        psum = stk.enter_context(tc.tile_pool(name="ps", bufs=8, space="PSUM"))
        work = stk.enter_context(tc.tile_pool(name="work", bufs=3))
        abp = stk.enter_context(tc.tile_pool(name="abp", bufs=5))
        smp = stk.enter_context(tc.tile_pool(name="smp", bufs=5))
        wld = stk.enter_context(tc.tile_pool(name="wld", bufs=2))
        strip = stk.enter_context(tc.tile_pool(name="strip", bufs=2))
        ainp = stk.enter_context(tc.tile_pool(name="ainp", bufs=5))
        rtp = stk.enter_context(tc.tile_pool(name="rtp", bufs=2))
        vpool = stk.enter_context(tc.tile_pool(name="vp", bufs=3))
        vkrd = stk.enter_context(tc.tile_pool(name="vkrd", bufs=3))
        akrd = stk.enter_context(tc.tile_pool(name="akrd", bufs=3))

        adjres = const_pool.tile([128, 3, 4, CH], bf16)
        ident = const_pool.tile([128, 128], f32)
        masks.make_identity(nc, ident[:])
        identb = const_pool.tile([128, 128], bf16)
        masks.make_identity(nc, identb[:])

        # ---- resident weights (bf16) and cluster data ----
        wsb = const_pool.tile([128, NLAYERS, DT, 512], bf16)
        for l in range(NLAYERS):
            for dd in range(DT):
                wt = wld.tile([128, 512], f32, tag="wload")
                nc.sync.dma_start(
                    wt[:], w_in.ap()[l, dd * 128:(dd + 1) * 128, :])
                nc.vector.tensor_copy(wsb[:, l, dd], wt[:])
        ctb = const_pool.tile([128, DT, K], bf16)
        for dd in range(DT):
            ctf = wld.tile([128, K], f32, tag="wload")
            nc.sync.dma_start(ctf[:], ct_in.ap()[dd * 128:(dd + 1) * 128, :])
            nc.vector.tensor_copy(ctb[:, dd], ctf[:])
        csq = const_pool.tile([128, K], f32)
        nc.sync.dma_start(csq[:], csq_in.ap())

        # ---- x -> rT0 (x^T, bf16, feature-major) ----
        rT = rtp.tile([128, DT, CH], bf16, name="rT0")
        for mt in range(MT):
            xt = work.tile([128, D], f32, tag="wk2")
            nc.sync.dma_start(xt[:], x_in.ap()[mt * 128:(mt + 1) * 128, :])
            xb = work.tile([128, D], bf16, tag="wk2")
            nc.scalar.activation(xb[:], xt[:], ACT.Copy)
            ps = psum.tile([128, 512], bf16, tag="ps")
            for dd in range(DT):
                nc.tensor.transpose(ps[:, dd * 128:(dd + 1) * 128],
                                    xb[:, dd * 128:(dd + 1) * 128], identb[:])
            nc.vector.tensor_copy(rT[:, :, mt * 128:(mt + 1) * 128],
                                  ps[:].rearrange("p (j m) -> p j m", j=4))

        # ---- layer 0 z = x @ W1 (unscaled), sent during the prologue ----
        for h in range(2):
            for mt in range(h * MT // 2, (h + 1) * MT // 2):
                zps = psum.tile([128, 512], f32, tag="ps", name=f"z0_{mt}")
                for dd in range(DT):
                    nc.tensor.matmul(zps[:],
                                     rT[:, dd, mt * 128:(mt + 1) * 128],
                                     wsb[:, 0, dd, :],
                                     start=(dd == 0), stop=(dd == DT - 1))
                vt = vpool.tile([128, 512], bf16, tag="v", name=f"v0_{mt}")
                nc.scalar.activation(vt[:], zps[:], ACT.Copy)
                nc.scalar.dma_start(
                    agin[0][h].ap()[(mt - h * MT // 2) * 128:
                                    (mt + 1 - h * MT // 2) * 128, :], vt[:])
            nc.gpsimd.collective_compute(
                "AllGather", mybir.AluOpType.bypass,
                replica_groups=[list(range(NCORES))],
                ins=[agin[0][h].ap()], outs=[agout[0][h].ap()])

        # ---- prologue: degree + adj^T (bf16) into DRAM scratch ----
        dega = const_pool.tile([128, MT], f32)
        nc.vector.memset(dega[:], 0.0)
        RES_SBS = {5: 0, 7: 1, 9: 2}
        for ks in range(16):           # 16 column superblocks (512 wide)
            res = RES_SBS.get(ks)
            if res is None:
                dst = strip.tile([128, 4, CH], bf16)
            else:
                dst = adjres[:, res]
            for tp in range(MT // 2):  # pairs of row strips
                ps = psum.tile([128, 1024], bf16, tag="ps")
                for u in range(2):
                    t = tp * 2 + u
                    ain = ainp.tile([128, 512], f32, tag="ain")
                    nc.sync.dma_start(
                        ain[:],
                        adj_in.ap()[t * 128:(t + 1) * 128,
                                    ks * 512:(ks + 1) * 512])
                    ab = abp.tile([128, 512], bf16, tag="abc")
                    part = smp.tile([128, 1], f32, tag="sm1")
                    nc.scalar.activation(ab[:], ain[:], ACT.Copy)
                    nc.vector.reduce_sum(part[:], ab[:],
                                         axis=mybir.AxisListType.X)
                    nc.vector.tensor_add(dega[:, t:t + 1], dega[:, t:t + 1],
                                         part[:])
                    for j in range(4):
                        nc.tensor.transpose(
                            ps[:, u * 512 + j * 128:u * 512 + (j + 1) * 128],
                            ab[:, j * 128:(j + 1) * 128],
                            identb[:])
                nc.vector.tensor_copy(
                    dst[:, :, tp * 256:(tp + 1) * 256].rearrange(
                        "p j (u m) -> p u j m", u=2),
                    ps[:].rearrange("p (u j m) -> p u j m", u=2, j=4))
            if res is None:
                for j in range(4):
                    nc.scalar.dma_start(
                        adjT.ap()[(ks * 4 + j) * 128:(ks * 4 + j + 1) * 128, :],
                        dst[:, j, :])

        # deg-derived row scales (all per local row, [128, MT] layout)
        dinv = const_pool.tile([128, MT], f32)
        nc.vector.reciprocal(dinv[:], dega[:])
        d1 = const_pool.tile([128, MT], f32)
        nc.scalar.activation(d1[:], dinv[:], ACT.Sqrt)
        m2d1 = const_pool.tile([128, MT], f32)
        nc.vector.tensor_scalar_mul(m2d1[:], d1[:], -2.0)
        # gather degrees; build d1g [128, KT] = rsqrt(deg) for every global node
        nc.scalar.dma_start(dgin.ap().rearrange("(t p) o -> p (t o)", p=128),
                            dega[:])
        nc.gpsimd.collective_compute(
            "AllGather", mybir.AluOpType.bypass,
            replica_groups=[list(range(NCORES))],
            ins=[dgin.ap()], outs=[dgout.ap()])
        degg = const_pool.tile([128, KT], f32)
        nc.sync.dma_start(degg[:],
                          dgout.ap().rearrange("(t p) o -> p (t o)", p=128))
        d1gi = const_pool.tile([128, KT], f32)
        nc.vector.reciprocal(d1gi[:], degg[:])
        d1g = const_pool.tile([128, KT], f32)
        nc.scalar.activation(d1g[:], d1gi[:], ACT.Sqrt)

        # ---- 4 GCN layers ----
        def mm_group(yps, vk, ak, mhs, start, stopg):
            """4 k-tiles of matmuls: yps[dd*2+mh] += vk.T @ ak slices."""
            for t in range(4):
                for dd in range(DT):
                    for mh in mhs:
                        nc.tensor.matmul(
                            yps[dd * 2 + mh][:],
                            vk[:, t, dd * 128:(dd + 1) * 128],
                            ak[:, t, mh * 512:(mh + 1) * 512],
                            start=(start and t == 0),
                            stop=(stopg and t == 3))

        def z_half(l_next, h, rT_cur, scale):
            """z + scaled bf16 send for m-half h feeding layer l_next's gather."""
            for mt in range(h * MT // 2, (h + 1) * MT // 2):
                zps = psum.tile([128, 512], f32, tag="ps",
                                name=f"z{l_next}_{mt}")
                for dd in range(DT):
                    nc.tensor.matmul(zps[:],
                                     rT_cur[:, dd, mt * 128:(mt + 1) * 128],
                                     wsb[:, l_next, dd, :],
                                     start=(dd == 0), stop=(dd == DT - 1))
                vt = vpool.tile([128, 512], bf16, tag="v",
                                name=f"v{l_next}_{mt}")
                nc.scalar.activation(vt[:], zps[:], ACT.Copy,
                                     scale=scale[:, mt:mt + 1])
                nc.scalar.dma_start(
                    agin[l_next][h].ap()[(mt - h * MT // 2) * 128:
                                         (mt + 1 - h * MT // 2) * 128, :],
                    vt[:])
            nc.gpsimd.collective_compute(
                "AllGather", mybir.AluOpType.bypass,
                replica_groups=[list(range(NCORES))],
                ins=[agin[l_next][h].ap()], outs=[agout[l_next][h].ap()])

        for l in range(NLAYERS):
            yps = []
            for _yi in range(8):
                ypt = psum.tile([128, 512], f32, tag="ps", name=f"yps{l}_{_yi}")
                yps.append(ypt)

            def load_vk(g, h, l=l, sfx=""):
                vk = vkrd.tile([128, 4, 512], bf16, tag="vk",
                               name=f"vk{l}_{h}_{g}{sfx}")
                nc.sync.dma_start(
                    vk[:],
                    agout[l][h].ap().rearrange("(r p) d -> p r d", p=128)[
                        :, g * 4:(g + 1) * 4, :])
                if l == 0:
                    for t in range(4):
                        kcol = g * 8 + h * 4 + t
                        nc.vector.tensor_scalar_mul(
                            vk[:, t, :], vk[:, t, :], d1g[:, kcol:kcol + 1])
                return vk

            def load_ak(g, h, l=l, sfx=""):
                ak = akrd.tile([128, 4, CH], bf16, tag="ak",
                               name=f"ak{l}_{h}_{g}{sfx}")
                nc.sync.dma_start(
                    ak[:],
                    adjT.ap().rearrange("(r p) m -> p r m", p=128)[
                        :, g * 8 + h * 4:g * 8 + h * 4 + 4, :])
                return ak

            # k-chunk 0 (first m-half rows of every core): both output halves
            for g in range(8):
                vk = load_vk(g, 0)
                ak = load_ak(g, 0)
                mm_group(yps, vk, ak, (0, 1), start=(g == 0), stopg=False)
            # k-chunk 1: output m-half 0 only
            for g in range(8):
                vk = load_vk(g, 1)
                ak = adjres[:, g - 2] if 2 <= g <= 4 else load_ak(g, 1)
                mm_group(yps, vk, ak, (0,), start=False, stopg=(g == 7))
            # first m-half of yT done: relu + feed next layer's first gather
            rT_next = rtp.tile([128, DT, CH], bf16, name=f"rT{l + 1}")
            for dd in range(DT):
                nc.scalar.activation(rT_next[:, dd, 0:512],
                                     yps[dd * 2 + 0][:], ACT.Relu)
            if l + 1 < NLAYERS:
                z_half(l + 1, 0, rT_next, dinv)
            # k-chunk 1, output m-half 1 (vk re-read; adjT mostly resident)
            for g in range(8):
                vk = load_vk(g, 1, sfx="b")
                ak = adjres[:, g - 2] if 2 <= g <= 4 else load_ak(g, 1, sfx="b")
                mm_group(yps, vk, ak, (1,), start=False, stopg=(g == 7))
            for dd in range(DT):
                nc.scalar.activation(rT_next[:, dd, 512:1024],
                                     yps[dd * 2 + 1][:], ACT.Relu)
            if l + 1 < NLAYERS:
                z_half(l + 1, 1, rT_next, dinv)
            rT = rT_next

        # ---- head: h = d1 * r ; q = student-t assignment ----
        for mt in range(MT):
            rps = psum.tile([128, 512], bf16, tag="ps", name=f"rps{mt}")
            for dd in range(DT):
                nc.tensor.transpose(rps[:, dd * 128:(dd + 1) * 128],
                                    rT[:, dd, mt * 128:(mt + 1) * 128],
                                    identb[:])
            ht = work.tile([128, D], f32, tag="wk2")
            nc.scalar.activation(ht[:], rps[:], ACT.Copy,
                                 scale=d1[:, mt:mt + 1])
            nc.sync.dma_start(h_out.ap()[mt * 128:(mt + 1) * 128, :], ht[:])
            hsqv = work.tile([128, D], f32, tag="wk2")
            nc.vector.tensor_mul(hsqv[:], ht[:], ht[:])
            hsq = smp.tile([128, 1], f32, tag="sm1")
            nc.vector.reduce_sum(hsq[:], hsqv[:], axis=mybir.AxisListType.X)
            gps = psum.tile([128, 512], f32, tag="ps", name=f"gps{mt}")
            for dd in range(DT):
                nc.tensor.matmul(gps[:, 0:K],
                                 rT[:, dd, mt * 128:(mt + 1) * 128],
                                 ctb[:, dd, :],
                                 start=(dd == 0), stop=(dd == DT - 1))
            t20 = smp.tile([128, K], f32, tag="sm")
            # sq = hsq + csq - 2*d1*G ; u = 1 + sq/2 ; q ~ u^-1.5 row-normalized
            nc.vector.tensor_scalar(t20[:], gps[:, 0:K], m2d1[:, mt:mt + 1],
                                    None, op0=mybir.AluOpType.mult)
            nc.vector.tensor_add(t20[:], t20[:], csq[:])
            nc.vector.tensor_scalar(t20[:], t20[:], hsq[:], None,
                                    op0=mybir.AluOpType.add)
            nc.vector.tensor_scalar(t20[:], t20[:], 0.5, 1.0,
                                    op0=mybir.AluOpType.mult,
                                    op1=mybir.AluOpType.add)
            rec = smp.tile([128, K], f32, tag="sm")
            nc.vector.reciprocal(rec[:], t20[:])
            sq = smp.tile([128, K], f32, tag="sm")
            nc.scalar.activation(sq[:], rec[:], ACT.Sqrt)
            qun = smp.tile([128, K], f32, tag="sm")
            nc.vector.tensor_mul(qun[:], rec[:], sq[:])
            srow = smp.tile([128, 1], f32, tag="sm1")
            nc.vector.reduce_sum(srow[:], qun[:], axis=mybir.AxisListType.X)
            rinv = smp.tile([128, 1], f32, tag="sm1")
            nc.vector.reciprocal(rinv[:], srow[:])
            qf = smp.tile([128, K], f32, tag="sm")
            nc.vector.tensor_scalar(qf[:], qun[:], rinv[:], None,
                                    op0=mybir.AluOpType.mult)
            nc.sync.dma_start(q_out.ap()[mt * 128:(mt + 1) * 128, :], qf[:])

    nc.compile()
    return nc


def _get_nc():
    if "nc" not in _CACHE:
        _CACHE["nc"] = _build()
    return _CACHE["nc"]


def make_in_maps(x, adj, w1, w2, w3, w4, cluster):
    w = np.stack([w1, w2, w3, w4]).astype(np.float32)
    ct = np.ascontiguousarray(cluster.T.astype(np.float32))
    csq = np.ascontiguousarray(
        np.broadcast_to((cluster.astype(np.float32) ** 2).sum(axis=1)[None, :],
                        (128, K))).astype(np.float32)
    in_maps = []
    for c in range(NCORES):
        shard = np.array(adj[c * CH:(c + 1) * CH, :], dtype=np.float32)
        idx = np.arange(CH)
        shard[idx, c * CH + idx] += 1.0   # fold adj_hat = adj + I into the shard
        in_maps.append({
            "adj": shard,
            "x": np.ascontiguousarray(x[c * CH:(c + 1) * CH].astype(np.float32)),
            "w": w, "ct": ct, "csq": csq,
        })
    return in_maps


def run(in_maps, trace=False):
    from concourse.bass_utils import run_bass_kernel_spmd
    return run_bass_kernel_spmd(_get_nc(), in_maps, list(range(NCORES)),
                                trace=trace)


def kernel(x, adj, w1, w2, w3, w4, cluster):
    res = run(make_in_maps(x, adj, w1, w2, w3, w4, cluster))
    h = np.concatenate([res.results[c]["h"] for c in range(NCORES)], axis=0)
    q = np.concatenate([res.results[c]["q"] for c in range(NCORES)], axis=0)
    return h, q
